# revision 19
# baseline (speedup 1.0000x reference)
"""Trainium2 Bass kernel for nn_MultiHeadAttention (B=4, T=2048, D=2048, H=16).

Sharding: tensor-parallel over heads; each of 8 NeuronCores owns 2 heads
(256 of the 2048 Q/K/V dims). Fully fused per-batch pipeline per core, all
matmuls bf16 (full 1 cycle/row PE rate; fp32 accumulate in PSUM):

  per batch b:
    proj:  kT, qT, v projected straight into SBUF; k/v also DMA'd out
           (bf16) as graded outputs.
    attn:  both heads interleaved chunk-pair by chunk-pair so the PE never
           waits on the softmax exp: scoresT = kT_chunk.T @ qT, one ACT exp
           per [128, 2, 512] pair (no max-subtraction -- logits are O(1) by
           construction), causal triangle masks on DVE, AV + ones-row
           denominator accumulated per chunk, 1/den via the fast Newton
           reciprocal on an SBUF copy, ctx normalized into SBUF.
    out:   out_partial = sum_h ctx_h.T @ WoT_h, DMA'd out bf16.

Host: out partials summed across cores in fp32; k/v slices concatenated.
"""

import os
import sys

import numpy as np

for _p in ("/opt/trn_rl_repo",):
    if _p not in sys.path and os.path.isdir(_p):
        sys.path.insert(0, _p)

B, T, D, H = 4, 2048, 2048, 16
HD = 128
N_CORES = 8
HPC = H // N_CORES          # heads per core
DPC = HPC * HD              # q/k/v dims per core
NTOK = B * T

P = 128
PT = 512                    # proj token tile
DK = D // P                 # 16 contraction chunks
NTT = NTOK // PT            # 16 token tiles overall
QT = 512                    # q-tile width
KC = 128                    # k-chunk
NQT = T // QT               # 4
TC = T // P                 # v token chunks per batch
NOD = D // QT               # 4 outproj column tiles

SCALE_EXP = 1.0 / float(np.sqrt(HD))

_CACHE = {}


def _build_module():
    import concourse.bass as bass  # noqa: F401
    import concourse.mybir as mybir
    from concourse import bacc
    import concourse.tile as tile

    F32 = mybir.dt.float32
    BF16 = mybir.dt.bfloat16
    AF = mybir.ActivationFunctionType

    nc = bacc.Bacc("TRN2", target_bir_lowering=False, debug=False)

    xA = nc.dram_tensor("xA", [NTT, P, DK, PT], BF16, kind="ExternalInput").ap()
    wqA = nc.dram_tensor("wqA", [P, DK, DPC], BF16, kind="ExternalInput").ap()
    wkA = nc.dram_tensor("wkA", [P, DK, DPC], BF16, kind="ExternalInput").ap()
    wvA = nc.dram_tensor("wvA", [P, DK, DPC], BF16, kind="ExternalInput").ap()
    woA = nc.dram_tensor("woA", [P, HPC, D], BF16, kind="ExternalInput").ap()
    maskA = nc.dram_tensor("maskA", [P, 2 * KC], BF16, kind="ExternalInput").ap()
    onesA = nc.dram_tensor("onesA", [P, P], BF16, kind="ExternalInput").ap()

    kT_out = nc.dram_tensor("kT_out", [DPC, NTOK], BF16, kind="ExternalOutput").ap()
    v_out = nc.dram_tensor("v_out", [NTOK, DPC], BF16, kind="ExternalOutput").ap()
    out_p = nc.dram_tensor("out_p", [NTOK, D], BF16, kind="ExternalOutput").ap()

    kT_v = kT_out.rearrange("(hc p) t -> p hc t", p=P)
    v_v = v_out.rearrange("(c p) m -> p c m", p=P)

    with tile.TileContext(nc) as tc:
        with (
            tc.tile_pool(name="w", bufs=1) as wp,
            tc.tile_pool(name="x", bufs=3) as xp,
            tc.tile_pool(name="qkv", bufs=2) as qp,
            tc.tile_pool(name="ctx", bufs=2) as cxp,
            tc.tile_pool(name="e", bufs=4) as ep,
            tc.tile_pool(name="r", bufs=3) as rp,
            tc.tile_pool(name="o", bufs=3) as op_,
            tc.tile_pool(name="ps_s", bufs=2, space="PSUM") as ps_s,
            tc.tile_pool(name="ps_c", bufs=2, space="PSUM") as ps_c,
            tc.tile_pool(name="ps_m", bufs=2, space="PSUM") as ps_m,
        ):
            wq_sb = wp.tile([P, DK, DPC], BF16, tag="wq")
            wk_sb = wp.tile([P, DK, DPC], BF16, tag="wk")
            wv_sb = wp.tile([P, DK, DPC], BF16, tag="wv")
            wo_sb = wp.tile([P, HPC, D], BF16, tag="wo")
            mask_sb = wp.tile([P, 2 * KC], BF16, tag="mask")
            ones_sb = wp.tile([P, P], BF16, tag="ones")
            nc.sync.dma_start(wk_sb[:], wkA)
            nc.sync.dma_start(wq_sb[:], wqA)
            nc.sync.dma_start(wv_sb[:], wvA)
            nc.sync.dma_start(wo_sb[:], woA)
            nc.sync.dma_start(mask_sb[:], maskA)
            nc.sync.dma_start(ones_sb[:], onesA)

            def load_x(b):
                t0 = b * (T // PT)
                xs = []
                for i in range(T // PT):
                    xlo = xp.tile([P, DK // 2, PT], BF16, tag="xb", name="xlo")
                    xhi = xp.tile([P, DK // 2, PT], BF16, tag="xb", name="xhi")
                    eng = nc.scalar if (b == 0 and i % 2 == 1) else nc.gpsimd
                    eng.dma_start(xlo[:], xA[t0 + i, :, :DK // 2, :])
                    eng.dma_start(xhi[:], xA[t0 + i, :, DK // 2:, :])
                    xs.append((xlo, xhi))
                return xs

            def xch(xi, dc):
                return xi[dc // (DK // 2)][:, dc % (DK // 2), :]

            # 4-slot PSUM ring for proj/outproj (ps_c's 2 banks are idle
            # outside attention, so borrow them to keep PE ahead of the
            # PSUM->SBUF evacuation copies)
            ps_rr = [0]

            def next_ps():
                ps_rr[0] ^= 1
                if ps_rr[0]:
                    return ps_m.tile([P, QT], F32, tag="m", name="psm")
                return ps_c.tile([P, QT], F32, tag="c", name="psc")

            def proj(b, xs, q_sb, k_sb, v_sb):
                for i, xi in enumerate(xs):
                    ts_ = slice(i * PT, (i + 1) * PT)
                    gts = slice(b * T + i * PT, b * T + (i + 1) * PT)
                    for hc in range(HPC):
                        ps = next_ps()
                        for dc in range(DK):
                            nc.tensor.matmul(
                                ps[:],
                                wk_sb[:, dc, hc * P:(hc + 1) * P],
                                xch(xi, dc),
                                start=(dc == 0), stop=(dc == DK - 1))
                        nc.vector.tensor_copy(k_sb[:, hc, ts_], ps[:])
                    nc.sync.dma_start(kT_v[:, :, gts], k_sb[:, :, ts_])
                    for hc in range(HPC):
                        ps = next_ps()
                        for dc in range(DK):
                            nc.tensor.matmul(
                                ps[:],
                                wq_sb[:, dc, hc * P:(hc + 1) * P],
                                xch(xi, dc),
                                start=(dc == 0), stop=(dc == DK - 1))
                        nc.vector.tensor_copy(q_sb[:, hc, ts_], ps[:])
                    for sub in range(PT // P):
                        c = i * (PT // P) + sub
                        ps = next_ps()
                        for dc in range(DK):
                            nc.tensor.matmul(
                                ps[:, :DPC],
                                xch(xi, dc)[:, sub * P:(sub + 1) * P],
                                wv_sb[:, dc, :],
                                start=(dc == 0), stop=(dc == DK - 1))
                        nc.vector.tensor_copy(v_sb[:, c, :], ps[:, :DPC])
                    nc.sync.dma_start(
                        v_v[:, b * TC + i * (PT // P):
                            b * TC + (i + 1) * (PT // P), :],
                        v_sb[:, i * (PT // P):(i + 1) * (PT // P), :])

            def outproj_qt(b, ctx_sb, qt):
                for tb in range(qt * (QT // P), (qt + 1) * (QT // P)):
                    ost = op_.tile([P, D], BF16, tag="ost")
                    for od in range(NOD):
                        ods = slice(od * QT, (od + 1) * QT)
                        pso = next_ps()
                        nc.tensor.matmul(
                            pso[:], ctx_sb[:, 0, tb * P:(tb + 1) * P],
                            wo_sb[:, 0, ods], start=True, stop=False)
                        nc.tensor.matmul(
                            pso[:], ctx_sb[:, 1, tb * P:(tb + 1) * P],
                            wo_sb[:, 1, ods], start=False, stop=True)
                        if od % 2 == 0:
                            nc.vector.tensor_copy(ost[:, ods], pso[:])
                        else:
                            nc.scalar.copy(ost[:, ods], pso[:])
                    t0 = b * T + tb * P
                    nc.sync.dma_start(out_p[t0:t0 + P, :], ost[:])

            def attn_both(b, q_sb, k_sb, v_sb, ctx_sb):
                # (qt, pair, is_first_pair_of_qt, is_last_pair_of_qt)
                tasks = [(qt, p, p == 0, p == 2 * (qt + 1) - 1)
                         for qt in range(NQT) for p in range(2 * (qt + 1))]
                st = {0: {}, 1: {}}
                pend = [None]

                def emit_S(h, idx):
                    qt, p, _, lastp = tasks[idx]
                    w = QT // 2 if lastp else QT
                    s = ps_s.tile([P, 2, QT], F32, tag="s")
                    qs = slice((qt + 1) * QT - w, (qt + 1) * QT)
                    for ci in range(2):
                        kc = 2 * p + ci
                        nc.tensor.matmul(
                            s[:, ci, :w],
                            k_sb[:, h, kc * KC:(kc + 1) * KC],
                            q_sb[:, h, qs],
                            start=True, stop=True)
                    st[h]['s'] = s

                def flush_pend():
                    # deferred 1/den + ctx normalize for the finished qt,
                    # emitted after the next pair's masks. Cheap copies
                    # evacuate the den/ctx PSUM banks first (so the next
                    # qt's accumulation never waits on the reciprocal
                    # chain); the recip + normalize then run on SBUF data
                    # entirely off the critical path.
                    if pend[0] is None:
                        return
                    qt0, saved = pend[0]
                    pend[0] = None
                    ev = {}
                    # at the last qt the next phase is outproj/proj (no
                    # exps), so ACT is idle: do the evacuation copies
                    # there to halve the PSUM-slot release latency
                    on_act = qt0 == NQT - 1
                    for h in (0, 1):
                        cps, dps = saved[h]
                        dsb = rp.tile([P, QT], F32, tag="dsb", name="dsb")
                        craw = rp.tile([P, QT], BF16, tag="craw", name="craw")
                        if on_act:
                            nc.scalar.copy(dsb[:], dps[:])
                            nc.scalar.copy(craw[:], cps[:])
                        else:
                            nc.vector.tensor_copy(dsb[:], dps[:])
                            nc.vector.tensor_copy(craw[:], cps[:])
                        ev[h] = (dsb, craw)
                    for h in (0, 1):
                        dsb, craw = ev[h]
                        r = rp.tile([P, QT], F32, tag="r", name="r")
                        nc.vector.reciprocal_approx_fast(r[:], dsb[:])
                        nc.vector.tensor_mul(
                            ctx_sb[:, h, qt0 * QT:(qt0 + 1) * QT],
                            craw[:], r[:])

                for h in (0, 1):
                    emit_S(h, 0)
                for i, (qt, p, first, lastp) in enumerate(tasks):
                    w = QT // 2 if lastp else QT
                    base = QT - w
                    for h in (0, 1):
                        e_pair = ep.tile([P, 2, QT], BF16, tag="e")
                        st[h]['e'] = e_pair
                        s = st[h].pop('s')
                        nc.scalar.activation(
                            e_pair[:, :, base:base + w], s[:, :, :w],
                            AF.Exp, scale=SCALE_EXP)
                        for ci in range(2):
                            j = 2 * p + ci - qt * (QT // KC)
                            if 0 <= j:
                                # triangle block only; the fully-masked
                                # columns below j*KC are skipped by the
                                # AV/den matmul ranges instead
                                nc.vector.tensor_mul(
                                    e_pair[:, ci, j * KC:(j + 1) * KC],
                                    e_pair[:, ci, j * KC:(j + 1) * KC],
                                    mask_sb[:, KC:])
                    flush_pend()
                    if i + 1 < len(tasks):
                        for h in (0, 1):
                            emit_S(h, i + 1)
                    if first:
                        for h in (0, 1):
                            cps = ps_c.tile([P, QT], F32, tag="c")
                            dps = ps_m.tile([P, QT], F32, tag="m")
                            st[h]['ctx'] = cps
                            st[h]['den'] = dps
                    for h in (0, 1):
                        e_pair = st[h]['e']
                        cps, dps = st[h]['ctx'], st[h]['den']
                        for ci in range(2):
                            kc = 2 * p + ci
                            j = kc - qt * (QT // KC)
                            lo = j * KC if j >= 0 else base
                            rhs = e_pair[:, ci, lo:QT]
                            stop = lastp and ci == 1
                            nc.tensor.matmul(
                                cps[:, lo:QT],
                                v_sb[:, kc, h * HD:(h + 1) * HD],
                                rhs, start=(kc == 0), stop=stop)
                            nc.tensor.matmul(
                                dps[:, lo:QT],
                                ones_sb[:, :],
                                rhs, start=(kc == 0), stop=stop)
                    if lastp:
                        pend[0] = (qt, {h: (st[h]['ctx'], st[h]['den'])
                                        for h in (0, 1)})
                flush_pend()

            xs = load_x(0)
            for b in range(B):
                xs_next = load_x(b + 1) if b + 1 < B else None
                q_sb = qp.tile([P, HPC, T], BF16, tag="q")
                k_sb = qp.tile([P, HPC, T], BF16, tag="k")
                v_sb = qp.tile([P, TC, DPC], BF16, tag="v")
                ctx_sb = cxp.tile([P, HPC, T], BF16, tag="ctx")
                proj(b, xs, q_sb, k_sb, v_sb)
                attn_both(b, q_sb, k_sb, v_sb, ctx_sb)
                for qt in range(NQT):
                    outproj_qt(b, ctx_sb, qt)
                xs = xs_next

    nc.compile()
    return nc


def _get_module():
    if "nc" not in _CACHE:
        _CACHE["nc"] = _build_module()
    return _CACHE["nc"]


def _host_inputs(x, Wq, Wk, Wv, Wo):
    import ml_dtypes

    bf16 = ml_dtypes.bfloat16

    x = np.asarray(x, np.float32)
    xT = np.ascontiguousarray(x.reshape(NTOK, D).T)           # [D, NTOK]
    # tile-major layout so every DMA is one big contiguous slab
    xA = np.ascontiguousarray(
        xT.reshape(DK, P, NTT, PT).transpose(2, 1, 0, 3)).astype(bf16)

    # m1[:, :128] is the "skip 128 then triangle" mask, m1[:, 128:] the
    # plain triangle
    m1 = np.zeros((P, 2 * KC), np.float32)
    for kk in range(P):
        m1[kk, KC + kk:] = 1.0

    shared = {
        "xA": xA,
        "maskA": m1.astype(bf16),
        "onesA": np.ones((P, P), bf16),
    }

    Wq = np.asarray(Wq, np.float32)
    Wk = np.asarray(Wk, np.float32)
    Wv = np.asarray(Wv, np.float32)
    Wo = np.asarray(Wo, np.float32)
    maps = []
    for c in range(N_CORES):
        sl = slice(c * DPC, (c + 1) * DPC)
        wqA = np.ascontiguousarray(
            Wq[sl, :].T.reshape(DK, P, DPC).transpose(1, 0, 2)).astype(bf16)
        wkA = np.ascontiguousarray(
            Wk[sl, :].T.reshape(DK, P, DPC).transpose(1, 0, 2)).astype(bf16)
        wvA = np.ascontiguousarray(
            Wv[sl, :].T.reshape(DK, P, DPC).transpose(1, 0, 2)).astype(bf16)
        woA = np.ascontiguousarray(
            Wo[:, sl].T.reshape(HPC, P, D).transpose(1, 0, 2)).astype(bf16)
        m = dict(shared)
        m.update({"wqA": wqA, "wkA": wkA, "wvA": wvA, "woA": woA})
        maps.append(m)
    return maps


def _run(x, Wq, Wk, Wv, Wo, bo, trace=False):
    from concourse import bass_utils

    nc = _get_module()
    in_maps = _host_inputs(x, Wq, Wk, Wv, Wo)
    res = bass_utils.run_bass_kernel_spmd(
        nc, in_maps, core_ids=list(range(N_CORES)), trace=trace)

    out = np.zeros((NTOK, D), np.float32)
    k = np.empty((NTOK, D), np.float32)
    v = np.empty((NTOK, D), np.float32)
    for c, r in enumerate(res.results):
        sl = slice(c * DPC, (c + 1) * DPC)
        out += np.asarray(r["out_p"], dtype=np.float32)
        k[:, sl] = np.asarray(r["kT_out"], dtype=np.float32).T
        v[:, sl] = np.asarray(r["v_out"], dtype=np.float32)
    out += np.asarray(bo, np.float32)[None, :]
    outs = (out.reshape(B, T, D), k.reshape(B, T, D), v.reshape(B, T, D))
    return outs, res


def kernel(x, Wq, Wk, Wv, Wo, bo):
    outs, _ = _run(x, Wq, Wk, Wv, Wo, bo, trace=False)
    return outs


# revision 21
# speedup vs baseline: 1.0063x; 1.0063x over previous
"""Trainium2 Bass kernel for nn_MultiHeadAttention (B=4, T=2048, D=2048, H=16).

Sharding: tensor-parallel over heads; each of 8 NeuronCores owns 2 heads
(256 of the 2048 Q/K/V dims). Fully fused per-batch pipeline per core, all
matmuls bf16 (full 1 cycle/row PE rate; fp32 accumulate in PSUM):

  per batch b:
    proj:  kT, qT, v projected straight into SBUF; k/v also DMA'd out
           (bf16) as graded outputs.
    attn:  both heads interleaved chunk-pair by chunk-pair so the PE never
           waits on the softmax exp: scoresT = kT_chunk.T @ qT, one ACT exp
           per [128, 2, 512] pair (no max-subtraction -- logits are O(1) by
           construction), causal triangle masks on DVE, AV + ones-row
           denominator accumulated per chunk, 1/den via the fast Newton
           reciprocal on an SBUF copy, ctx normalized into SBUF.
    out:   out_partial = sum_h ctx_h.T @ WoT_h, DMA'd out bf16.

Host: out partials summed across cores in fp32; k/v slices concatenated.
"""

import os
import sys

import numpy as np

for _p in ("/opt/trn_rl_repo",):
    if _p not in sys.path and os.path.isdir(_p):
        sys.path.insert(0, _p)

B, T, D, H = 4, 2048, 2048, 16
HD = 128
N_CORES = 8
HPC = H // N_CORES          # heads per core
DPC = HPC * HD              # q/k/v dims per core
NTOK = B * T

P = 128
PT = 512                    # proj token tile
DK = D // P                 # 16 contraction chunks
NTT = NTOK // PT            # 16 token tiles overall
QT = 512                    # q-tile width
KC = 128                    # k-chunk
NQT = T // QT               # 4
TC = T // P                 # v token chunks per batch
NOD = D // QT               # 4 outproj column tiles

SCALE_EXP = 1.0 / float(np.sqrt(HD))

_CACHE = {}


def _build_module():
    import concourse.bass as bass  # noqa: F401
    import concourse.mybir as mybir
    from concourse import bacc
    import concourse.tile as tile

    F32 = mybir.dt.float32
    BF16 = mybir.dt.bfloat16
    AF = mybir.ActivationFunctionType

    nc = bacc.Bacc("TRN2", target_bir_lowering=False, debug=False)

    xA = nc.dram_tensor("xA", [NTT, P, DK, PT], BF16, kind="ExternalInput").ap()
    wqA = nc.dram_tensor("wqA", [P, DK, DPC], BF16, kind="ExternalInput").ap()
    wkA = nc.dram_tensor("wkA", [P, DK, DPC], BF16, kind="ExternalInput").ap()
    wvA = nc.dram_tensor("wvA", [P, DK, DPC], BF16, kind="ExternalInput").ap()
    woA = nc.dram_tensor("woA", [P, HPC, D], BF16, kind="ExternalInput").ap()
    maskA = nc.dram_tensor("maskA", [P, 2 * KC], BF16, kind="ExternalInput").ap()
    onesA = nc.dram_tensor("onesA", [P, P], BF16, kind="ExternalInput").ap()

    kT_out = nc.dram_tensor("kT_out", [DPC, NTOK], BF16, kind="ExternalOutput").ap()
    v_out = nc.dram_tensor("v_out", [NTOK, DPC], BF16, kind="ExternalOutput").ap()
    out_p = nc.dram_tensor("out_p", [NTOK, D], BF16, kind="ExternalOutput").ap()

    kT_v = kT_out.rearrange("(hc p) t -> p hc t", p=P)
    v_v = v_out.rearrange("(c p) m -> p c m", p=P)

    with tile.TileContext(nc) as tc:
        with (
            tc.tile_pool(name="w", bufs=1) as wp,
            tc.tile_pool(name="x", bufs=3) as xp,
            tc.tile_pool(name="qkv", bufs=2) as qp,
            tc.tile_pool(name="ctx", bufs=2) as cxp,
            tc.tile_pool(name="e", bufs=4) as ep,
            tc.tile_pool(name="r", bufs=3) as rp,
            tc.tile_pool(name="o", bufs=4) as op_,
            tc.tile_pool(name="ps_s", bufs=2, space="PSUM") as ps_s,
            tc.tile_pool(name="ps_c", bufs=2, space="PSUM") as ps_c,
            tc.tile_pool(name="ps_m", bufs=2, space="PSUM") as ps_m,
        ):
            wq_sb = wp.tile([P, DK, DPC], BF16, tag="wq")
            wk_sb = wp.tile([P, DK, DPC], BF16, tag="wk")
            wv_sb = wp.tile([P, DK, DPC], BF16, tag="wv")
            wo_sb = wp.tile([P, HPC, D], BF16, tag="wo")
            mask_sb = wp.tile([P, 2 * KC], BF16, tag="mask")
            ones_sb = wp.tile([P, P], BF16, tag="ones")
            nc.sync.dma_start(wk_sb[:], wkA)
            nc.sync.dma_start(wq_sb[:], wqA)
            nc.sync.dma_start(wv_sb[:], wvA)
            nc.sync.dma_start(wo_sb[:], woA)
            nc.sync.dma_start(mask_sb[:], maskA)
            nc.sync.dma_start(ones_sb[:], onesA)

            def load_x(b):
                t0 = b * (T // PT)
                xs = []
                for i in range(T // PT):
                    xlo = xp.tile([P, DK // 2, PT], BF16, tag="xb", name="xlo")
                    xhi = xp.tile([P, DK // 2, PT], BF16, tag="xb", name="xhi")
                    nc.gpsimd.dma_start(xlo[:], xA[t0 + i, :, :DK // 2, :])
                    nc.gpsimd.dma_start(xhi[:], xA[t0 + i, :, DK // 2:, :])
                    xs.append((xlo, xhi))
                return xs

            def xch(xi, dc):
                return xi[dc // (DK // 2)][:, dc % (DK // 2), :]

            # 4-slot PSUM ring for proj/outproj (ps_c's 2 banks are idle
            # outside attention, so borrow them to keep PE ahead of the
            # PSUM->SBUF evacuation copies)
            ps_rr = [0]

            def next_ps():
                ps_rr[0] = (ps_rr[0] + 1) % 3
                if ps_rr[0] == 0:
                    return ps_m.tile([P, QT], F32, tag="m", name="psm")
                if ps_rr[0] == 1:
                    return ps_c.tile([P, QT], F32, tag="c", name="psc")
                # bank-aligned half of an idle attention score slot
                pss = ps_s.tile([P, 2, QT], F32, tag="s", name="pss")
                return pss[:, 0, :]

            def proj(b, xs, q_sb, k_sb, v_sb):
                for i, xi in enumerate(xs):
                    ts_ = slice(i * PT, (i + 1) * PT)
                    gts = slice(b * T + i * PT, b * T + (i + 1) * PT)
                    for hc in range(HPC):
                        ps = next_ps()
                        for dc in range(DK):
                            nc.tensor.matmul(
                                ps[:],
                                wk_sb[:, dc, hc * P:(hc + 1) * P],
                                xch(xi, dc),
                                start=(dc == 0), stop=(dc == DK - 1))
                        nc.vector.tensor_copy(k_sb[:, hc, ts_], ps[:])
                    nc.sync.dma_start(kT_v[:, :, gts], k_sb[:, :, ts_])
                    for hc in range(HPC):
                        ps = next_ps()
                        for dc in range(DK):
                            nc.tensor.matmul(
                                ps[:],
                                wq_sb[:, dc, hc * P:(hc + 1) * P],
                                xch(xi, dc),
                                start=(dc == 0), stop=(dc == DK - 1))
                        nc.vector.tensor_copy(q_sb[:, hc, ts_], ps[:])
                    for sub in range(PT // P):
                        c = i * (PT // P) + sub
                        ps = next_ps()
                        for dc in range(DK):
                            nc.tensor.matmul(
                                ps[:, :DPC],
                                xch(xi, dc)[:, sub * P:(sub + 1) * P],
                                wv_sb[:, dc, :],
                                start=(dc == 0), stop=(dc == DK - 1))
                        nc.vector.tensor_copy(v_sb[:, c, :], ps[:, :DPC])
                    nc.sync.dma_start(
                        v_v[:, b * TC + i * (PT // P):
                            b * TC + (i + 1) * (PT // P), :],
                        v_sb[:, i * (PT // P):(i + 1) * (PT // P), :])

            def outproj_qt(b, ctx_sb, qt):
                for tb in range(qt * (QT // P), (qt + 1) * (QT // P)):
                    ost = op_.tile([P, D], BF16, tag="ost")
                    for od in range(NOD):
                        ods = slice(od * QT, (od + 1) * QT)
                        pso = next_ps()
                        nc.tensor.matmul(
                            pso[:], ctx_sb[:, 0, tb * P:(tb + 1) * P],
                            wo_sb[:, 0, ods], start=True, stop=False)
                        nc.tensor.matmul(
                            pso[:], ctx_sb[:, 1, tb * P:(tb + 1) * P],
                            wo_sb[:, 1, ods], start=False, stop=True)
                        if od % 2 == 0:
                            nc.vector.tensor_copy(ost[:, ods], pso[:])
                        else:
                            nc.scalar.copy(ost[:, ods], pso[:])
                    t0 = b * T + tb * P
                    nc.sync.dma_start(out_p[t0:t0 + P, :], ost[:])

            def attn_both(b, q_sb, k_sb, v_sb, ctx_sb):
                # (qt, pair, is_first_pair_of_qt, is_last_pair_of_qt)
                tasks = [(qt, p, p == 0, p == 2 * (qt + 1) - 1)
                         for qt in range(NQT) for p in range(2 * (qt + 1))]
                st = {0: {}, 1: {}}
                pend = [None]

                def emit_S(h, idx):
                    qt, p, _, lastp = tasks[idx]
                    w = QT // 2 if lastp else QT
                    s = ps_s.tile([P, 2, QT], F32, tag="s")
                    qs = slice((qt + 1) * QT - w, (qt + 1) * QT)
                    for ci in range(2):
                        kc = 2 * p + ci
                        nc.tensor.matmul(
                            s[:, ci, :w],
                            k_sb[:, h, kc * KC:(kc + 1) * KC],
                            q_sb[:, h, qs],
                            start=True, stop=True)
                    st[h]['s'] = s

                def flush_pend():
                    # deferred 1/den + ctx normalize for the finished qt,
                    # emitted after the next pair's masks. Cheap copies
                    # evacuate the den/ctx PSUM banks first (so the next
                    # qt's accumulation never waits on the reciprocal
                    # chain); the recip + normalize then run on SBUF data
                    # entirely off the critical path.
                    if pend[0] is None:
                        return
                    qt0, saved = pend[0]
                    pend[0] = None
                    ev = {}
                    for h in (0, 1):
                        cps, dps = saved[h]
                        dsb = rp.tile([P, QT], F32, tag="dsb", name="dsb")
                        nc.vector.tensor_copy(dsb[:], dps[:])
                        craw = rp.tile([P, QT], BF16, tag="craw", name="craw")
                        nc.vector.tensor_copy(craw[:], cps[:])
                        ev[h] = (dsb, craw)
                    for h in (0, 1):
                        dsb, craw = ev[h]
                        r = rp.tile([P, QT], F32, tag="r", name="r")
                        nc.vector.reciprocal_approx_fast(r[:], dsb[:])
                        nc.vector.tensor_mul(
                            ctx_sb[:, h, qt0 * QT:(qt0 + 1) * QT],
                            craw[:], r[:])

                for h in (0, 1):
                    emit_S(h, 0)
                for i, (qt, p, first, lastp) in enumerate(tasks):
                    w = QT // 2 if lastp else QT
                    base = QT - w
                    for h in (0, 1):
                        e_pair = ep.tile([P, 2, QT], BF16, tag="e")
                        st[h]['e'] = e_pair
                        s = st[h].pop('s')
                        nc.scalar.activation(
                            e_pair[:, :, base:base + w], s[:, :, :w],
                            AF.Exp, scale=SCALE_EXP)
                        for ci in range(2):
                            j = 2 * p + ci - qt * (QT // KC)
                            if 0 <= j:
                                # triangle block only; the fully-masked
                                # columns below j*KC are skipped by the
                                # AV/den matmul ranges instead
                                nc.vector.tensor_mul(
                                    e_pair[:, ci, j * KC:(j + 1) * KC],
                                    e_pair[:, ci, j * KC:(j + 1) * KC],
                                    mask_sb[:, KC:])
                    flush_pend()
                    if i + 1 < len(tasks):
                        for h in (0, 1):
                            emit_S(h, i + 1)
                    if first:
                        for h in (0, 1):
                            cps = ps_c.tile([P, QT], F32, tag="c")
                            dps = ps_m.tile([P, QT], F32, tag="m")
                            st[h]['ctx'] = cps
                            st[h]['den'] = dps
                    for h in (0, 1):
                        e_pair = st[h]['e']
                        cps, dps = st[h]['ctx'], st[h]['den']
                        for ci in range(2):
                            kc = 2 * p + ci
                            j = kc - qt * (QT // KC)
                            lo = j * KC if j >= 0 else base
                            rhs = e_pair[:, ci, lo:QT]
                            stop = lastp and ci == 1
                            nc.tensor.matmul(
                                cps[:, lo:QT],
                                v_sb[:, kc, h * HD:(h + 1) * HD],
                                rhs, start=(kc == 0), stop=stop)
                            nc.tensor.matmul(
                                dps[:, lo:QT],
                                ones_sb[:, :],
                                rhs, start=(kc == 0), stop=stop)
                    if lastp:
                        pend[0] = (qt, {h: (st[h]['ctx'], st[h]['den'])
                                        for h in (0, 1)})
                flush_pend()

            xs = load_x(0)
            for b in range(B):
                xs_next = load_x(b + 1) if b + 1 < B else None
                q_sb = qp.tile([P, HPC, T], BF16, tag="q")
                k_sb = qp.tile([P, HPC, T], BF16, tag="k")
                v_sb = qp.tile([P, TC, DPC], BF16, tag="v")
                ctx_sb = cxp.tile([P, HPC, T], BF16, tag="ctx")
                proj(b, xs, q_sb, k_sb, v_sb)
                attn_both(b, q_sb, k_sb, v_sb, ctx_sb)
                for qt in range(NQT):
                    outproj_qt(b, ctx_sb, qt)
                xs = xs_next

    nc.compile()
    return nc


def _get_module():
    if "nc" not in _CACHE:
        _CACHE["nc"] = _build_module()
    return _CACHE["nc"]


def _host_inputs(x, Wq, Wk, Wv, Wo):
    import ml_dtypes

    bf16 = ml_dtypes.bfloat16

    x = np.asarray(x, np.float32)
    xT = np.ascontiguousarray(x.reshape(NTOK, D).T)           # [D, NTOK]
    # tile-major layout so every DMA is one big contiguous slab
    xA = np.ascontiguousarray(
        xT.reshape(DK, P, NTT, PT).transpose(2, 1, 0, 3)).astype(bf16)

    # m1[:, :128] is the "skip 128 then triangle" mask, m1[:, 128:] the
    # plain triangle
    m1 = np.zeros((P, 2 * KC), np.float32)
    for kk in range(P):
        m1[kk, KC + kk:] = 1.0

    shared = {
        "xA": xA,
        "maskA": m1.astype(bf16),
        "onesA": np.ones((P, P), bf16),
    }

    Wq = np.asarray(Wq, np.float32)
    Wk = np.asarray(Wk, np.float32)
    Wv = np.asarray(Wv, np.float32)
    Wo = np.asarray(Wo, np.float32)
    maps = []
    for c in range(N_CORES):
        sl = slice(c * DPC, (c + 1) * DPC)
        wqA = np.ascontiguousarray(
            Wq[sl, :].T.reshape(DK, P, DPC).transpose(1, 0, 2)).astype(bf16)
        wkA = np.ascontiguousarray(
            Wk[sl, :].T.reshape(DK, P, DPC).transpose(1, 0, 2)).astype(bf16)
        wvA = np.ascontiguousarray(
            Wv[sl, :].T.reshape(DK, P, DPC).transpose(1, 0, 2)).astype(bf16)
        woA = np.ascontiguousarray(
            Wo[:, sl].T.reshape(HPC, P, D).transpose(1, 0, 2)).astype(bf16)
        m = dict(shared)
        m.update({"wqA": wqA, "wkA": wkA, "wvA": wvA, "woA": woA})
        maps.append(m)
    return maps


def _run(x, Wq, Wk, Wv, Wo, bo, trace=False):
    from concourse import bass_utils

    nc = _get_module()
    in_maps = _host_inputs(x, Wq, Wk, Wv, Wo)
    res = bass_utils.run_bass_kernel_spmd(
        nc, in_maps, core_ids=list(range(N_CORES)), trace=trace)

    out = np.zeros((NTOK, D), np.float32)
    k = np.empty((NTOK, D), np.float32)
    v = np.empty((NTOK, D), np.float32)
    for c, r in enumerate(res.results):
        sl = slice(c * DPC, (c + 1) * DPC)
        out += np.asarray(r["out_p"], dtype=np.float32)
        k[:, sl] = np.asarray(r["kT_out"], dtype=np.float32).T
        v[:, sl] = np.asarray(r["v_out"], dtype=np.float32)
    out += np.asarray(bo, np.float32)[None, :]
    outs = (out.reshape(B, T, D), k.reshape(B, T, D), v.reshape(B, T, D))
    return outs, res


def kernel(x, Wq, Wk, Wv, Wo, bo):
    outs, _ = _run(x, Wq, Wk, Wv, Wo, bo, trace=False)
    return outs
